# revision 1
# baseline (speedup 1.0000x reference)
"""Trainium2 Bass kernel for CRF mean-field iteration (nn_CRF).

Math (derived from the reference):
    comp = -I  =>  each iteration is   x <- x0 + w * smooth(softmax(x, C))
    output = log_softmax(x_final, C)
where smooth = per-channel separable 11-tap Gaussian blur over H then W
('same' zero padding, center tap zeroed, per-sample spacing).

Strategy (per core, 2 samples, pure data parallel over batch):
  - State layout in SBUF: xbuf[p, c, j, w] = x[c, 128*j + p, w]
    (h on partitions in 3 chunks of 128; free dim = (channel, chunk, width),
    so per-channel and whole-sample DMA views stay 3-dim contiguous).
  - Softmax: ACT exp (in-place), channel-sum via a GPSIMD pairwise tree,
    r = 1/S via the fast DVE Newton reciprocal, p = e*r as per-channel
    contiguous DVE multiplies.
  - Conv along H as matmul with the *data as the stationary operand*
    (out1[w,h'] = sum_h p[h,w]*Th[h,h']), which lands transposed in PSUM.
    Conv along W the same way on out1, landing back in [h', w'] layout.
    Th/Tw are banded symmetric Toeplitz matrices built on the host from the
    runtime spacing/theta inputs; smoothness_weight is folded into Tw.
    Band structure: for contraction chunk j only output cols
    [128j-5, 128j+133) are touched; PSUM has_written semantics handle the
    overlap (accumulate where written, overwrite where not).
  - x_new = x0 + s fused as one DVE tensor_add reading PSUM directly.
"""

import sys

if "/opt/trn_rl_repo" not in sys.path:
    sys.path.insert(0, "/opt/trn_rl_repo")

from contextlib import ExitStack

import numpy as np

import concourse.bass as bass
import concourse.tile as tile
from concourse import bacc, mybir

F32 = mybir.dt.float32
AF = mybir.ActivationFunctionType

B, C, H, W = 16, 16, 384, 384
N_CORES = 8
BPC = B // N_CORES  # samples per core
N_ITER = 5
FS = 11
HALF = FS // 2  # 5
P = 128
NCH = H // P  # 3 h-chunks
NCW = W // P  # 3 w-chunks


def _band(j, n):
    """Output-column range touched by contraction chunk j of a banded T."""
    return max(0, P * j - HALF), min(n, P * j + P + HALF)


def _crf_kernel(ctx, tc, out_d, x_in, th_in, tw_in, n_samples, n_iter, full_j0):
    nc = tc.nc

    state = ctx.enter_context(tc.tile_pool(name="state", bufs=1))
    mats = ctx.enter_context(tc.tile_pool(name="mats", bufs=1))
    stage = ctx.enter_context(tc.tile_pool(name="stage", bufs=2))
    small = ctx.enter_context(tc.tile_pool(name="small", bufs=1))
    psum = ctx.enter_context(tc.tile_pool(name="psum", bufs=2, space="PSUM"))

    xbuf = state.tile([P, C, NCH, W], F32, tag="xbuf")
    x0buf = state.tile([P, C, NCH, W], F32, tag="x0buf")

    for b in range(n_samples):
        # ---- load inputs for this sample ----
        # One DMA for the whole sample: fewer HWDGE-queue semaphores for
        # downstream waits (TT sync-wait ISA limit) and better DMA batching.
        nc.sync.dma_start(
            out=x0buf[:],
            in_=x_in[b].rearrange("c (j p) w -> p c j w", p=P),
        )
        th_sb = mats.tile([P, NCH, H], F32, tag="th")
        tw_sb = mats.tile([P, NCW, W], F32, tag="tw")
        nc.sync.dma_start(out=th_sb[:], in_=th_in[b].rearrange("(j p) n -> p j n", p=P))
        nc.sync.dma_start(out=tw_sb[:], in_=tw_in[b].rearrange("(j p) n -> p j n", p=P))

        # Softmax emission helpers. Emitted interleaved with the previous
        # iteration's conv loop so each engine's program order (~= Tile
        # schedule order) lets exps/partial-sums run DURING the conv phase.
        def emit_exp_cg(src, vts, cg):
            sl = slice(4 * cg, 4 * cg + 4)
            for j in range(NCH):
                nc.scalar.activation(
                    out=xbuf[:, sl, j], in_=src[:, sl, j], func=AF.Exp
                )
                ut = small.tile([P, 2, W], F32, tag="tu")
                nc.gpsimd.tensor_add(
                    ut[:], xbuf[:, 4 * cg : 4 * cg + 2, j],
                    xbuf[:, 4 * cg + 2 : 4 * cg + 4, j],
                )
                nc.vector.tensor_add(
                    vts[j][:, cg : cg + 1], ut[:, 0:1], ut[:, 1:2]
                )

        def emit_s_and_p(vts, sball, rall):
            for j in range(NCH):
                wt = small.tile([P, 2, W], F32, tag="twv")
                nc.gpsimd.tensor_add(wt[:, 0:1], vts[j][:, 0:1], vts[j][:, 1:2])
                nc.gpsimd.tensor_add(wt[:, 1:2], vts[j][:, 2:3], vts[j][:, 3:4])
                nc.vector.tensor_add(
                    sball[:, j : j + 1], wt[:, 0:1], wt[:, 1:2]
                )
                nc.vector.reciprocal_approx_fast(rall[:, j], sball[:, j])
            for c in range(C):
                nc.vector.tensor_mul(out=xbuf[:, c], in0=xbuf[:, c], in1=rall[:])

        def new_smax_tiles():
            sball = small.tile([P, NCH, W], F32, tag="S")
            rall = small.tile([P, NCH, W], F32, tag="r")
            vts = [small.tile([P, 4, W], F32, tag=f"tv{j}", name=f"vt{j}") for j in range(NCH)]
            return sball, rall, vts

        # Prologue: softmax of iteration 0 from x0.
        sball, rall, vts = new_smax_tiles()
        for cg in range(4):
            emit_exp_cg(x0buf, vts, cg)
        emit_s_and_p(vts, sball, rall)

        for it in range(n_iter):
            last = it == n_iter - 1
            if not last:
                nball, nrall, nvts = new_smax_tiles()
            # ---- smoothing convs + fused x-update, per channel ----
            for c in range(C):
                pA = psum.tile([P, NCH, 512], F32, tag="ps")
                for m in range(NCW):
                    for j in range(NCH):
                        # CoreSim needs j==0 to cover the full width (its
                        # pending-zero model can't mix accumulate/overwrite in
                        # one matmul); HW has_written handles the banded
                        # overlap per element, so skip the extra columns there.
                        n0, n1 = (0, H) if (j == 0 and full_j0) else _band(j, H)
                        nc.tensor.matmul(
                            pA[:, m, n0:n1],
                            lhsT=xbuf[:, c, j, m * P : (m + 1) * P],
                            rhs=th_sb[:, j, n0:n1],
                            start=(j == 0),
                            stop=(j == NCH - 1),
                        )
                o1 = stage.tile([P, NCW, H], F32, tag="o1")
                nc.scalar.copy(out=o1[:], in_=pA[:, :, 0:H])
                pB = psum.tile([P, NCH, 512], F32, tag="ps")
                for m in range(NCH):
                    for j in range(NCW):
                        n0, n1 = (0, W) if (j == 0 and full_j0) else _band(j, W)
                        nc.tensor.matmul(
                            pB[:, m, n0:n1],
                            lhsT=o1[:, j, m * P : (m + 1) * P],
                            rhs=tw_sb[:, j, n0:n1],
                            start=(j == 0),
                            stop=(j == NCW - 1),
                        )
                nc.vector.tensor_add(
                    out=xbuf[:, c], in0=x0buf[:, c], in1=pB[:, :, 0:W]
                )
                # Next iteration's softmax for this channel group becomes
                # ready as soon as its 4 channels' updates land — emit here
                # so it overlaps the remaining channels' convs.
                if not last and c % 4 == 3:
                    emit_exp_cg(xbuf, nvts, c // 4)
            if not last:
                emit_s_and_p(nvts, nball, nrall)
                sball, rall, vts = nball, nrall, nvts

        # ---- final log_softmax: out = x - log(sum_c exp(x)) ----
        # Dedicated exp scratch: reusing x0buf here made the NEXT sample's x0
        # DMA wait for the whole final pass (measured 130us PE stall).
        lball = small.tile([P, NCH, W], F32, tag="r")
        for j in range(NCH):
            vt = small.tile([P, 4, W], F32, tag="tv")
            for cg in range(4):
                sl = slice(4 * cg, 4 * cg + 4)
                fe = stage.tile([P, 4, W], F32, tag="o1")
                nc.scalar.activation(
                    out=fe[:], in_=xbuf[:, sl, j], func=AF.Exp
                )
                ut = small.tile([P, 2, W], F32, tag="tu")
                nc.gpsimd.tensor_add(ut[:], fe[:, 0:2], fe[:, 2:4])
                nc.vector.tensor_add(vt[:, cg : cg + 1], ut[:, 0:1], ut[:, 1:2])
            wt = small.tile([P, 2, W], F32, tag="twv")
            nc.gpsimd.tensor_add(wt[:, 0:1], vt[:, 0:1], vt[:, 1:2])
            nc.gpsimd.tensor_add(wt[:, 1:2], vt[:, 2:3], vt[:, 3:4])
            sb = small.tile([P, 1, W], F32, tag="S")
            nc.vector.tensor_add(sb[:], wt[:, 0:1], wt[:, 1:2])
            nc.scalar.activation(out=lball[:, j], in_=sb[:, 0], func=AF.Ln)
        for c in range(C):
            nc.vector.tensor_sub(out=xbuf[:, c], in0=xbuf[:, c], in1=lball[:])
        nc.sync.dma_start(
            out=out_d[b].rearrange("c (j p) w -> p c j w", p=P),
            in_=xbuf[:],
        )


def build_nc(n_samples=BPC, n_iter=N_ITER, full_j0=False):
    # Bacc (not plain Bass): its compile() pass legalizes multi-wait
    # instructions via InstEventSemaphore — walrus caps regular instructions
    # at ONE sync wait.
    nc = bacc.Bacc()
    x_in = nc.dram_tensor("x", [n_samples, C, H, W], F32, kind="ExternalInput")
    th_in = nc.dram_tensor("th", [n_samples, H, H], F32, kind="ExternalInput")
    tw_in = nc.dram_tensor("tw", [n_samples, W, W], F32, kind="ExternalInput")
    out_d = nc.dram_tensor("out", [n_samples, C, H, W], F32, kind="ExternalOutput")
    with tile.TileContext(nc) as tc:
        with ExitStack() as ctx:
            _crf_kernel(ctx, tc, out_d, x_in, th_in, tw_in, n_samples, n_iter, full_j0)
    nc.finalize()
    return nc


def make_toeplitz(spacing, inv_theta, size, weight=1.0):
    """Banded symmetric Toeplitz matrix for the 1D 'same' correlation."""
    d = spacing * np.arange(-(FS // 2), FS // 2 + 1, dtype=np.float32)
    k = np.exp(-((d * inv_theta) ** 2) / 2.0).astype(np.float32)
    k[FS // 2] = 0.0
    t = np.zeros((size, size), dtype=np.float32)
    for tap in range(FS):
        off = tap - FS // 2  # out[h] += k[tap] * x[h + off]
        idx = np.arange(max(0, -off), min(size, size - off))
        t[idx + off, idx] = k[tap]
    return (t * weight).astype(np.float32)


def host_prep(x, spatial_spacings, smoothness_weight, inv_smoothness_theta):
    """Build per-sample Th (H-conv) and weight-scaled Tw (W-conv) matrices."""
    w = float(np.asarray(smoothness_weight))
    th = np.stack(
        [
            make_toeplitz(float(spatial_spacings[b, 0]), float(inv_smoothness_theta[0]), H)
            for b in range(x.shape[0])
        ]
    )
    tw = np.stack(
        [
            make_toeplitz(
                float(spatial_spacings[b, 1]), float(inv_smoothness_theta[1]), W, weight=w
            )
            for b in range(x.shape[0])
        ]
    )
    return th, tw


_NC_CACHE = {}


def kernel(x, spatial_spacings, smoothness_weight, inv_smoothness_theta):
    from concourse.bass_utils import run_bass_kernel_spmd

    x = np.ascontiguousarray(np.asarray(x), dtype=np.float32)
    spatial_spacings = np.asarray(spatial_spacings, dtype=np.float32)
    th, tw = host_prep(x, spatial_spacings, smoothness_weight, inv_smoothness_theta)

    key = (BPC, N_ITER)
    if key not in _NC_CACHE:
        _NC_CACHE[key] = build_nc(BPC, N_ITER)
    nc = _NC_CACHE[key]

    core_ids = list(range(N_CORES))
    in_maps = []
    for i in core_ids:
        sl = slice(i * BPC, (i + 1) * BPC)
        in_maps.append({"x": x[sl], "th": th[sl], "tw": tw[sl]})
    res = run_bass_kernel_spmd(nc, in_maps, core_ids)
    out = np.concatenate([res.results[i]["out"] for i in core_ids], axis=0)
    return out.astype(np.float32)


if __name__ == "__main__":
    rng = np.random.default_rng(0)
    x = rng.standard_normal((B, C, H, W), dtype=np.float32)
    out = kernel(
        x,
        np.ones((B, 2), np.float32),
        np.float32(1.0),
        np.ones((2,), np.float32),
    )
    print(out.shape, out.dtype)



# revision 3
# speedup vs baseline: 1.7074x; 1.7074x over previous
"""Trainium2 Bass kernel for CRF mean-field iteration (nn_CRF).

Math (derived from the reference):
    comp = -I  =>  each iteration is   x <- x0 + w * smooth(softmax(x, C))
    output = log_softmax(x_final, C)
where smooth = per-channel separable 11-tap Gaussian blur over H then W
('same' zero padding, center tap zeroed, per-sample spacing).

v2 strategy (per core, 2 samples, pure data parallel over batch):
  - All conv matmuls in bf16 (1 cyc/row on PE vs 4 for fp32; FWL weight
    loads). Th/Tw built and cast to bf16 on the host; x cast to bf16 on
    the host so SBUF state is bf16 and DMA halves.
  - The x-update (x_new = x0 + s) rides on the PE: an identity-matmul
    accumulates x0 into the W-conv PSUM group (identity first with
    start=True, conv matmuls accumulate on top). PSUM then holds x_new.
  - ACT exp reads that PSUM bank directly (ScE is fastest from PSUM) and
    writes e = exp(x_new) to SBUF in bf16. No separate x materialization
    except on the last iteration (xf copied into the retired x0 slot).
  - Channel sum S via a pairwise tree in bf16 (DVE 2x mode; first-level
    pairs mostly on GPSIMD to offload DVE), r = 1/S via fast reciprocal,
    p = e*r in-place at DVE 2x.
  - o1 (mid-conv PSUM evacuation) split between ACT and DVE per channel
    to balance the two PSUM-capable engines.
  - Final: out = xf - ln(S5), subtraction on GPSIMD (SBUF only), DMA out
    per channel.
"""

import sys

if "/opt/trn_rl_repo" not in sys.path:
    sys.path.insert(0, "/opt/trn_rl_repo")

from contextlib import ExitStack

import numpy as np

import concourse.bass as bass
import concourse.tile as tile
from concourse import bacc, mybir

F32 = mybir.dt.float32
BF16 = mybir.dt.bfloat16
AF = mybir.ActivationFunctionType

B, C, H, W = 16, 16, 384, 384
N_CORES = 8
BPC = B // N_CORES  # samples per core
N_ITER = 5
FS = 11
HALF = FS // 2  # 5
P = 128
NCH = H // P  # 3 h-chunks
NCW = W // P  # 3 w-chunks

# --- engine assignment tuning knobs ---
# o1 PSUM->SBUF copy: channels with (c < O1_ACT_CH) go to ACT, rest DVE.
O1_ACT_CH = 7
# xf PSUM->SBUF copy on last iteration: channels with (c < XF_ACT_CH) -> ACT.
XF_ACT_CH = 6
# tree level-1 pairs: pairs [0, TREE_L1_POOL) on gpsimd, rest on DVE.
TREE_L1_POOL = 7
# final (xf - L) subtraction engine: "pool" or "vector"
FINAL_SUB_POOL = True


def _band(j, n):
    """Output-column range touched by contraction chunk j of a banded T."""
    return max(0, P * j - HALF), min(n, P * j + P + HALF)


def _crf_kernel(ctx, tc, out_d, x_in, th_in, tw_in, id_in, n_samples, n_iter):
    nc = tc.nc

    state = ctx.enter_context(tc.tile_pool(name="state", bufs=2))
    shared = ctx.enter_context(tc.tile_pool(name="shared", bufs=1))
    mats = ctx.enter_context(tc.tile_pool(name="mats", bufs=2))
    o1p = ctx.enter_context(tc.tile_pool(name="o1p", bufs=2))
    outp = ctx.enter_context(tc.tile_pool(name="outp", bufs=3))
    psum = ctx.enter_context(tc.tile_pool(name="psum", bufs=2, space="PSUM"))

    # shared across samples
    ep = shared.tile([P, C, NCH, W], BF16, tag="ep")  # exp / p (in-place)
    t1 = shared.tile([P, 8, NCH, W], BF16, tag="t1")  # tree scratch
    sS = shared.tile([P, NCH, W], F32, tag="S")
    rr = shared.tile([P, NCH, W], F32, tag="r")
    rb = shared.tile([P, NCH, W], BF16, tag="rb")
    lnb = shared.tile([P, NCH, W], F32, tag="lnb")
    ident = shared.tile([P, P], BF16, tag="ident")
    nc.sync.dma_start(out=ident[:], in_=id_in[:])

    def emit_tree_partial(c):
        """Progressive pairwise channel-sum emissions after exp(c) lands."""
        if c >= 8:
            i = c - 8
            # level 1: t1[i] = ep[i] + ep[i+8]
            eng = nc.gpsimd if i < TREE_L1_POOL else nc.vector
            eng.tensor_add(t1[:, i], ep[:, i], ep[:, c])
            # level 2: t1[i] += t1[i+4] for i in 0..3, ready when both exist
            if i >= 4:
                nc.vector.tensor_add(t1[:, i - 4], t1[:, i - 4], t1[:, i])

    def emit_tree_tail():
        """Levels 3/4 + reciprocal + bf16 cast of r."""
        nc.vector.tensor_add(t1[:, 0], t1[:, 0], t1[:, 2])
        nc.vector.tensor_add(t1[:, 1], t1[:, 1], t1[:, 3])
        nc.vector.tensor_add(sS[:], t1[:, 0], t1[:, 1])
        nc.vector.reciprocal_approx_fast(out=rr[:], in_=sS[:])
        nc.vector.tensor_copy(rb[:], rr[:])

    def emit_pmul():
        for c in range(C):
            nc.vector.tensor_mul(out=ep[:, c], in0=ep[:, c], in1=rb[:])

    for b in range(n_samples):
        x0 = state.tile([P, C, NCH, W], BF16, tag="x0", name=f"x0_{b}")
        nc.sync.dma_start(
            out=x0[:],
            in_=x_in[b].rearrange("c (j p) w -> p c j w", p=P),
        )
        th_sb = mats.tile([P, NCH, H], BF16, tag="th", name=f"th_{b}")
        tw_sb = mats.tile([P, NCW, W], BF16, tag="tw", name=f"tw_{b}")
        nc.sync.dma_start(out=th_sb[:], in_=th_in[b].rearrange("(j p) n -> p j n", p=P))
        nc.sync.dma_start(out=tw_sb[:], in_=tw_in[b].rearrange("(j p) n -> p j n", p=P))

        # ---- prologue: p1 = softmax(x0) ----
        for c in range(C):
            nc.scalar.activation(out=ep[:, c], in_=x0[:, c], func=AF.Exp)
            emit_tree_partial(c)
        emit_tree_tail()
        emit_pmul()

        # ---- iterations ----
        for it in range(n_iter):
            last = it == n_iter - 1
            for c in range(C):
                # H-conv: out1[w, h'] = sum_h p[h, w] Th[h, h']  (PSUM pA)
                pA = psum.tile([P, NCH, 512], F32, tag="ps")
                for m in range(NCW):
                    for j in range(NCH):
                        n0, n1 = _band(j, H)
                        nc.tensor.matmul(
                            pA[:, m, n0:n1],
                            lhsT=ep[:, c, j, m * P : (m + 1) * P],
                            rhs=th_sb[:, j, n0:n1],
                            start=(j == 0),
                            stop=(j == NCH - 1),
                        )
                o1 = o1p.tile([P, NCW, H], BF16, tag="o1")
                if c < O1_ACT_CH:
                    nc.scalar.copy(out=o1[:], in_=pA[:, :, 0:H])
                else:
                    nc.vector.tensor_copy(o1[:], pA[:, :, 0:H])
                # W-conv + x0 accumulation: pB = x0[c] + o1 @ Tw
                pB = psum.tile([P, NCH, 512], F32, tag="ps")
                for m in range(NCH):
                    nc.tensor.matmul(
                        pB[:, m, 0:W],
                        lhsT=ident[:],
                        rhs=x0[:, c, m, :],
                        start=True,
                        stop=False,
                    )
                    for j in range(NCW):
                        n0, n1 = _band(j, W)
                        nc.tensor.matmul(
                            pB[:, m, n0:n1],
                            lhsT=o1[:, j, m * P : (m + 1) * P],
                            rhs=tw_sb[:, j, n0:n1],
                            start=False,
                            stop=(j == NCW - 1),
                        )
                # evacuate PSUM: e = exp(x_new) (+ xf copy on last iter)
                nc.scalar.activation(out=ep[:, c], in_=pB[:, :, 0:W], func=AF.Exp)
                if last:
                    # keep x_final; x0[c] slot is dead after the identity-adds
                    if c < XF_ACT_CH:
                        nc.scalar.copy(out=x0[:, c], in_=pB[:, :, 0:W])
                    else:
                        nc.vector.tensor_copy(x0[:, c], pB[:, :, 0:W])
                emit_tree_partial(c)
            emit_tree_tail()
            if not last:
                emit_pmul()

        # ---- final: out = xf - ln(S5) ----
        nc.scalar.activation(out=lnb[:], in_=sS[:], func=AF.Ln)
        for c in range(C):
            ot = outp.tile([P, NCH, W], F32, tag="ot")
            if FINAL_SUB_POOL:
                nc.gpsimd.tensor_sub(ot[:], x0[:, c], lnb[:])
            else:
                nc.vector.tensor_sub(ot[:], x0[:, c], lnb[:])
            nc.sync.dma_start(
                out=out_d[b, c].rearrange("(j p) w -> p j w", p=P),
                in_=ot[:],
            )


def build_nc(n_samples=BPC, n_iter=N_ITER, full_j0=False):
    # Bacc (not plain Bass): its compile() pass legalizes multi-wait
    # instructions via InstEventSemaphore — walrus caps regular instructions
    # at ONE sync wait.
    nc = bacc.Bacc()
    x_in = nc.dram_tensor("x", [n_samples, C, H, W], BF16, kind="ExternalInput")
    th_in = nc.dram_tensor("th", [n_samples, H, H], BF16, kind="ExternalInput")
    tw_in = nc.dram_tensor("tw", [n_samples, W, W], BF16, kind="ExternalInput")
    id_in = nc.dram_tensor("ident", [P, P], BF16, kind="ExternalInput")
    out_d = nc.dram_tensor("out", [n_samples, C, H, W], F32, kind="ExternalOutput")
    with tile.TileContext(nc) as tc:
        with ExitStack() as ctx:
            _crf_kernel(ctx, tc, out_d, x_in, th_in, tw_in, id_in, n_samples, n_iter)
    nc.finalize()
    return nc


def make_toeplitz(spacing, inv_theta, size, weight=1.0):
    """Banded symmetric Toeplitz matrix for the 1D 'same' correlation."""
    d = spacing * np.arange(-(FS // 2), FS // 2 + 1, dtype=np.float32)
    k = np.exp(-((d * inv_theta) ** 2) / 2.0).astype(np.float32)
    k[FS // 2] = 0.0
    t = np.zeros((size, size), dtype=np.float32)
    for tap in range(FS):
        off = tap - FS // 2  # out[h] += k[tap] * x[h + off]
        idx = np.arange(max(0, -off), min(size, size - off))
        t[idx + off, idx] = k[tap]
    return (t * weight).astype(np.float32)


def to_bf16(a):
    import ml_dtypes

    return np.asarray(a, dtype=np.float32).astype(ml_dtypes.bfloat16)


def host_prep(x, spatial_spacings, smoothness_weight, inv_smoothness_theta):
    """Build per-sample Th (H-conv) and weight-scaled Tw (W-conv) matrices."""
    w = float(np.asarray(smoothness_weight))
    th = np.stack(
        [
            make_toeplitz(float(spatial_spacings[b, 0]), float(inv_smoothness_theta[0]), H)
            for b in range(x.shape[0])
        ]
    )
    tw = np.stack(
        [
            make_toeplitz(
                float(spatial_spacings[b, 1]), float(inv_smoothness_theta[1]), W, weight=w
            )
            for b in range(x.shape[0])
        ]
    )
    return to_bf16(th), to_bf16(tw)


_NC_CACHE = {}


def kernel(x, spatial_spacings, smoothness_weight, inv_smoothness_theta):
    from concourse.bass_utils import run_bass_kernel_spmd

    x = to_bf16(x)
    spatial_spacings = np.asarray(spatial_spacings, dtype=np.float32)
    th, tw = host_prep(x, spatial_spacings, smoothness_weight, inv_smoothness_theta)
    ident = to_bf16(np.eye(P, dtype=np.float32))

    key = (BPC, N_ITER)
    if key not in _NC_CACHE:
        _NC_CACHE[key] = build_nc(BPC, N_ITER)
    nc = _NC_CACHE[key]

    core_ids = list(range(N_CORES))
    in_maps = []
    for i in core_ids:
        sl = slice(i * BPC, (i + 1) * BPC)
        in_maps.append({"x": x[sl], "th": th[sl], "tw": tw[sl], "ident": ident})
    res = run_bass_kernel_spmd(nc, in_maps, core_ids)
    out = np.concatenate([res.results[i]["out"] for i in core_ids], axis=0)
    return out.astype(np.float32)


if __name__ == "__main__":
    rng = np.random.default_rng(0)
    x = rng.standard_normal((B, C, H, W), dtype=np.float32)
    out = kernel(
        x,
        np.ones((B, 2), np.float32),
        np.float32(1.0),
        np.ones((2,), np.float32),
    )
    print(out.shape, out.dtype)


# revision 4
# speedup vs baseline: 2.4208x; 1.4178x over previous
"""Trainium2 Bass kernel for CRF mean-field iteration (nn_CRF).

Math (derived from the reference):
    comp = -I  =>  each iteration is   x <- x0 + w * smooth(softmax(x, C))
    output = log_softmax(x_final, C)
where smooth = per-channel separable 11-tap Gaussian blur over H then W
('same' zero padding, center tap zeroed, per-sample spacing).

v3 strategy (per core, 2 samples in channel-level lockstep):
  - All conv matmuls in bf16 (1 cyc/row on PE; FWL weight loads). Th/Tw
    and x cast to bf16 on the host.
  - x-update rides on the PE: an identity-matmul accumulates x0 into the
    W-conv PSUM group (identity first with start=True, convs accumulate
    on top). PSUM then holds x_new; ACT exp reads it directly and writes
    e = exp(x_new) to SBUF in bf16.
  - Channel sum S also rides on the PE: 16 identity-matmuls accumulate
    e[c] into a PSUM slot (exact fp32 sum). r = 1/S via fast reciprocal
    (PSUM source), cast to bf16, p = e*r in-place at DVE 2x.
  - Two samples interleaved per channel: each sample owns a 1-deep
    [P,3,512] PSUM ring (pA -> pB -> ... -> S), so engine program order
    always has the other sample's independent work available; PE never
    idles long enough for HAM to re-throttle.
  - o1/xf PSUM evacuations split between ACT and DVE by knobs; final
    out = xf - ln(S5) subtraction on GPSIMD (SBUF-only); DMA out per
    channel.
"""

import sys

if "/opt/trn_rl_repo" not in sys.path:
    sys.path.insert(0, "/opt/trn_rl_repo")

from contextlib import ExitStack

import numpy as np

import concourse.bass as bass
import concourse.tile as tile
from concourse import bacc, mybir

F32 = mybir.dt.float32
BF16 = mybir.dt.bfloat16
AF = mybir.ActivationFunctionType

B, C, H, W = 16, 16, 384, 384
N_CORES = 8
BPC = B // N_CORES  # samples per core
N_ITER = 5
FS = 11
HALF = FS // 2  # 5
P = 128
NCH = H // P  # 3 h-chunks
NCW = W // P  # 3 w-chunks

# --- engine assignment knobs ---
# o1 PSUM->SBUF copy: channels with (c % 16) < O1_ACT_CH go to ACT, rest DVE.
O1_ACT_CH = 7
# xf PSUM->SBUF copy on last iteration: c < XF_ACT_CH -> ACT, rest DVE.
XF_ACT_CH = 8
# p = e*r multiply: channels with c >= C - PMUL_POOL_CH go to gpsimd.
PMUL_POOL_CH = 2


def _band(j, n):
    return max(0, P * j - HALF), min(n, P * j + P + HALF)


class SampleCtx:
    """Per-sample tiles and emission state."""

    def __init__(self, tc, pools, b):
        nc = tc.nc
        state, shared, mats, psum = pools
        self.b = b
        self.x0 = state.tile([P, C, NCH, W], BF16, tag=f"x0_{b}")
        self.ep = state.tile([P, C, NCH, W], BF16, tag=f"ep_{b}")
        self.rr = state.tile([P, NCH, W], F32, tag=f"rr_{b}")
        self.rb = state.tile([P, NCH, W], BF16, tag=f"rb_{b}")
        self.th = mats.tile([P, NCH, H], BF16, tag=f"th_{b}")
        self.tw = mats.tile([P, NCW, W], BF16, tag=f"tw_{b}")
        self.psum = psum  # per-sample pool
        self.pA = None
        self.pB = None


def _crf_kernel(ctx, tc, out_d, x_in, th_in, tw_in, id_in, n_samples, n_iter):
    nc = tc.nc
    assert n_samples == 2

    state = ctx.enter_context(tc.tile_pool(name="state", bufs=1))
    shared = ctx.enter_context(tc.tile_pool(name="shared", bufs=1))
    mats = ctx.enter_context(tc.tile_pool(name="mats", bufs=1))
    o1p = ctx.enter_context(tc.tile_pool(name="o1p", bufs=4))
    outp = ctx.enter_context(tc.tile_pool(name="outp", bufs=4))
    psA = ctx.enter_context(tc.tile_pool(name="psA", bufs=1, space="PSUM"))
    psB = ctx.enter_context(tc.tile_pool(name="psB", bufs=1, space="PSUM"))

    ident = shared.tile([P, P], BF16, tag="ident")
    nc.sync.dma_start(out=ident[:], in_=id_in[:])
    lnb = shared.tile([P, NCH, W], F32, tag="lnb")

    S = [SampleCtx(tc, (state, shared, mats, ps), b)
         for b, ps in zip(range(n_samples), (psA, psB))]

    for s in S:
        b = s.b
        nc.sync.dma_start(
            out=s.x0[:], in_=x_in[b].rearrange("c (j p) w -> p c j w", p=P)
        )
        nc.sync.dma_start(out=s.th[:], in_=th_in[b].rearrange("(j p) n -> p j n", p=P))
        nc.sync.dma_start(out=s.tw[:], in_=tw_in[b].rearrange("(j p) n -> p j n", p=P))

    # --- emission helpers (all take a SampleCtx) ---
    def emit_hconv(s, c):
        s.pA = s.psum.tile([P, NCH, 512], F32, tag=f"ps{s.b}")
        for m in range(NCW):
            for j in range(NCH):
                n0, n1 = _band(j, H)
                nc.tensor.matmul(
                    s.pA[:, m, n0:n1],
                    lhsT=s.ep[:, c, j, m * P : (m + 1) * P],
                    rhs=s.th[:, j, n0:n1],
                    start=(j == 0),
                    stop=(j == NCH - 1),
                )

    def emit_o1(s, c):
        o1 = o1p.tile([P, NCW, H], BF16, tag="o1")
        if c < O1_ACT_CH:
            nc.scalar.copy(out=o1[:], in_=s.pA[:, :, 0:H])
        else:
            nc.vector.tensor_copy(o1[:], s.pA[:, :, 0:H])
        s.o1 = o1

    def emit_wconv(s, c):
        s.pB = s.psum.tile([P, NCH, 512], F32, tag=f"ps{s.b}")
        for m in range(NCH):
            nc.tensor.matmul(
                s.pB[:, m, 0:W],
                lhsT=ident[:],
                rhs=s.x0[:, c, m, :],
                start=True,
                stop=False,
            )
            for j in range(NCW):
                n0, n1 = _band(j, W)
                nc.tensor.matmul(
                    s.pB[:, m, n0:n1],
                    lhsT=s.o1[:, j, m * P : (m + 1) * P],
                    rhs=s.tw[:, j, n0:n1],
                    start=False,
                    stop=(j == NCW - 1),
                )

    def emit_exp_psum(s, c, last):
        nc.scalar.activation(out=s.ep[:, c], in_=s.pB[:, :, 0:W], func=AF.Exp)
        if last:
            if c < XF_ACT_CH:
                nc.scalar.copy(out=s.x0[:, c], in_=s.pB[:, :, 0:W])
            else:
                nc.vector.tensor_copy(s.x0[:, c], s.pB[:, :, 0:W])

    def emit_ssum(s):
        """S = sum_c e[c] via PE identity-matmuls into the PSUM ring."""
        s.pS = s.psum.tile([P, NCH, 512], F32, tag=f"ps{s.b}")
        for c in range(C):
            for j in range(NCH):
                nc.tensor.matmul(
                    s.pS[:, j, 0:W],
                    lhsT=ident[:],
                    rhs=s.ep[:, c, j, :],
                    start=(c == 0),
                    stop=(c == C - 1),
                )

    def emit_recip(s):
        nc.vector.reciprocal_approx_fast(out=s.rr[:], in_=s.pS[:, :, 0:W])
        nc.vector.tensor_copy(s.rb[:], s.rr[:])

    def emit_pmul(s, c):
        eng = nc.gpsimd if c >= C - PMUL_POOL_CH else nc.vector
        eng.tensor_mul(out=s.ep[:, c], in0=s.ep[:, c], in1=s.rb[:])

    # --- prologue: p1 = softmax(x0), interleaved across samples ---
    for c in range(C):
        for s in S:
            nc.scalar.activation(out=s.ep[:, c], in_=s.x0[:, c], func=AF.Exp)
    for s in S:
        emit_ssum(s)
    for s in S:
        emit_recip(s)
    for c in range(C):
        for s in S:
            emit_pmul(s, c)

    # --- iterations ---
    for it in range(n_iter):
        last = it == n_iter - 1
        for c in range(C):
            for s in S:
                emit_hconv(s, c)
            for s in S:
                emit_o1(s, c)
            for s in S:
                emit_wconv(s, c)
            for s in S:
                emit_exp_psum(s, c, last)
        for s in S:
            emit_ssum(s)
        if not last:
            for s in S:
                emit_recip(s)
            for c in range(C):
                for s in S:
                    emit_pmul(s, c)

    # --- final: out = xf - ln(S5) ---
    for s in S:
        nc.scalar.activation(out=lnb[:], in_=s.pS[:, :, 0:W], func=AF.Ln)
        for c in range(C):
            ot = outp.tile([P, NCH, W], F32, tag="ot")
            nc.gpsimd.tensor_sub(ot[:], s.x0[:, c], lnb[:])
            nc.sync.dma_start(
                out=out_d[s.b, c].rearrange("(j p) w -> p j w", p=P),
                in_=ot[:],
            )


def build_nc(n_samples=BPC, n_iter=N_ITER, full_j0=False):
    nc = bacc.Bacc()
    x_in = nc.dram_tensor("x", [n_samples, C, H, W], BF16, kind="ExternalInput")
    th_in = nc.dram_tensor("th", [n_samples, H, H], BF16, kind="ExternalInput")
    tw_in = nc.dram_tensor("tw", [n_samples, W, W], BF16, kind="ExternalInput")
    id_in = nc.dram_tensor("ident", [P, P], BF16, kind="ExternalInput")
    out_d = nc.dram_tensor("out", [n_samples, C, H, W], F32, kind="ExternalOutput")
    with tile.TileContext(nc) as tc:
        with ExitStack() as ctx:
            _crf_kernel(ctx, tc, out_d, x_in, th_in, tw_in, id_in, n_samples, n_iter)
    nc.finalize()
    return nc


def make_toeplitz(spacing, inv_theta, size, weight=1.0):
    d = spacing * np.arange(-(FS // 2), FS // 2 + 1, dtype=np.float32)
    k = np.exp(-((d * inv_theta) ** 2) / 2.0).astype(np.float32)
    k[FS // 2] = 0.0
    t = np.zeros((size, size), dtype=np.float32)
    for tap in range(FS):
        off = tap - FS // 2
        idx = np.arange(max(0, -off), min(size, size - off))
        t[idx + off, idx] = k[tap]
    return (t * weight).astype(np.float32)


def to_bf16(a):
    import ml_dtypes

    return np.asarray(a, dtype=np.float32).astype(ml_dtypes.bfloat16)


def host_prep(x, spatial_spacings, smoothness_weight, inv_smoothness_theta):
    w = float(np.asarray(smoothness_weight))
    th = np.stack(
        [
            make_toeplitz(float(spatial_spacings[b, 0]), float(inv_smoothness_theta[0]), H)
            for b in range(x.shape[0])
        ]
    )
    tw = np.stack(
        [
            make_toeplitz(
                float(spatial_spacings[b, 1]), float(inv_smoothness_theta[1]), W, weight=w
            )
            for b in range(x.shape[0])
        ]
    )
    return to_bf16(th), to_bf16(tw)


_NC_CACHE = {}


def kernel(x, spatial_spacings, smoothness_weight, inv_smoothness_theta):
    from concourse.bass_utils import run_bass_kernel_spmd

    x = to_bf16(x)
    spatial_spacings = np.asarray(spatial_spacings, dtype=np.float32)
    th, tw = host_prep(x, spatial_spacings, smoothness_weight, inv_smoothness_theta)
    ident = to_bf16(np.eye(P, dtype=np.float32))

    key = (BPC, N_ITER)
    if key not in _NC_CACHE:
        _NC_CACHE[key] = build_nc(BPC, N_ITER)
    nc = _NC_CACHE[key]

    core_ids = list(range(N_CORES))
    in_maps = []
    for i in core_ids:
        sl = slice(i * BPC, (i + 1) * BPC)
        in_maps.append({"x": x[sl], "th": th[sl], "tw": tw[sl], "ident": ident})
    res = run_bass_kernel_spmd(nc, in_maps, core_ids)
    out = np.concatenate([res.results[i]["out"] for i in core_ids], axis=0)
    return out.astype(np.float32)


if __name__ == "__main__":
    rng = np.random.default_rng(0)
    x = rng.standard_normal((B, C, H, W), dtype=np.float32)
    out = kernel(
        x,
        np.ones((B, 2), np.float32),
        np.float32(1.0),
        np.ones((2,), np.float32),
    )
    print(out.shape, out.dtype)
